# revision 54
# baseline (speedup 1.0000x reference)
"""NodeEquilibriumLoss Trainium2 kernel (fp8 DoubleRow version).

residual[b] = (EA[b] * e[b]) @ S - q[b] - r[b];  out = mean(residual^2)

S[elem, 2*node+c] = sum_k [elem_ids[k]==elem][node_ids[k]==node] * vecs[k, c]
is the fixed sparse linear map implementing the reference's gather+scatter-add.

Sharding: data-parallel over batch, 8 cores x 512 rows. Per core:
  - S is held in fp8e4 with row mapping sigma(e) = kt*128 + p, packed as
    int16 pairs (cols 2n/2n+1 share one int16 slot) and built on device from
    compact (index, packed-value) tables via 32 gpsimd.local_scatter calls
    (~26us on Pool, fully overlapped with the input DMA stream). One tile
    per (kt2, h) window keeps the matmul rhs dependencies call-precise.
  - per 128-row batch tile: ax = EA*e in bf16 (DVE), 16 PE chunk-transposes
    into one bf16 PSUM bank (start only on chunk 0 so the bank zero-region
    survives), Act evacuates to SBUF with a cast to fp8 -> axT[p, kt*128+b].
  - 8 fp8 DoubleRow matmuls (K=256 each: lhsT dim1 = adjacent kt pair)
    accumulate each [128, 512] output block in PSUM; a copy frees the bank
    (Act for early tiles, DVE late), then d -= q, d -= r and a
    square+accumulate into acc[:, col] stream behind the q/r loads, spread
    across DVE/Pool/Act as three parallel tail chains (the terminal block's
    chain is the shortest: DVE sub + DVE square).
  - DMA order: one table blob (scatter idx/val + transpose identity) first
    on the SP queue, then EA/e, then q/r halves; the last tile's q/r arrive
    as 512/256-col quarters and the terminal block squares as two 256-wide
    DVE chains into separate acc columns, so the post-last-load chain is
    minimal. All loads are chain-ready when scheduled, keeping the global
    DMA stream dense (~47.5us, the memory roofline).
  - output: [128, 17] per-partition partial sums; host reduces in fp64.
"""

import numpy as np
import ml_dtypes

B, NE, NN, E2 = 4096, 2048, 1024, 4096
N2 = 2 * NN
NCORES = 8
SHARD = B // NCORES   # 512
BT = 128              # batch rows per tile
NT = SHARD // BT      # 4 batch tiles per core
KT2 = 8               # DoubleRow contraction tiles (K=256 each)
NBLK = 4              # output column blocks of 512 per batch tile
NIDX = 16             # padded nonzeros per (elem row, col half)

_CACHE = {}


def _build_bass(reps=1, out_pad=0):
    # reps>1 repeats the whole computation (idempotently) inside one NEFF;
    # out_pad widens the output tensor so the HLO (and the libneuronxla NEFF
    # cache key, which ignores the embedded BIR) differs between variants.
    # Both are used only by timing harnesses.
    from concourse import bacc
    import concourse.mybir as mybir
    import concourse.tile as tile

    f32 = mybir.dt.float32
    fp8 = mybir.dt.float8e4
    i16 = mybir.dt.int16
    DR = mybir.MatmulPerfMode.DoubleRow
    Square = mybir.ActivationFunctionType.Square

    nc = bacc.Bacc("TRN2", target_bir_lowering=False, debug=False,
                   num_devices=NCORES)
    EA = nc.dram_tensor("EA", [SHARD, NE], f32, kind="ExternalInput").ap()
    ee = nc.dram_tensor("e", [SHARD, NE], f32, kind="ExternalInput").ap()
    qq = nc.dram_tensor("q", [SHARD, N2], f32, kind="ExternalInput").ap()
    rr = nc.dram_tensor("r", [SHARD, N2], f32, kind="ExternalInput").ap()
    # one blob per partition: sidx [512 i16] | sval [512 i16] | ident
    # [128 bf16 = 128 i16] -> 1152 i16; a single DMA feeds the scatter
    # chain + transposes with minimal head latency
    tabs = nc.dram_tensor("tabs", [128, 1152], i16,
                          kind="ExternalInput").ap()
    NCOL = NT * NBLK + 1  # terminal block contributes two half-columns
    out = nc.dram_tensor("out", [128, NCOL + out_pad], f32,
                         kind="ExternalOutput").ap()

    HW2 = N2 // 2   # 1024 cols per q/r half-load

    with tile.TileContext(nc) as tc:
        with (
            tc.tile_pool(name="sconst", bufs=1) as sconst,
            tc.tile_pool(name="io", bufs=2) as io,
            tc.tile_pool(name="work", bufs=2) as work,
            tc.tile_pool(name="ps", bufs=8, space="PSUM") as psp,
        ):
            for _rep in range(reps):
                # --- tables + S build (Pool; overlaps everything) ---
                tab_t = sconst.tile([128, 1152], i16)
                # On the SP queue (fastest DMA issue path) ahead of EA/e so
                # the scatter chain starts immediately.
                with tc.high_priority():
                    nc.sync.dma_start(out=tab_t, in_=tabs)
                idx_t = tab_t[:, 0:512].rearrange(
                    "p (a b c d) -> p a b c d", a=KT2, b=2, c=2, d=NIDX)
                val_t = tab_t[:, 512:1024].rearrange(
                    "p (a b c d) -> p a b c d", a=KT2, b=2, c=2, d=NIDX)
                id_t = tab_t[:, 1024:1152].bitcast(mybir.dt.bfloat16)
                # One tile per (kt2, h) window [p, i, slot] so the matmul's
                # rhs dependency is exactly that window's two scatter calls
                # (cross-dtype subtile range tracking is imprecise).
                S_tiles = {}
                for h in range(2):
                    for kt2 in range(KT2):
                        st = sconst.tile([128, 2, 512], i16,
                                         name=f"S_{kt2}_{h}_{_rep}")
                        for i in range(2):
                            nc.gpsimd.local_scatter(
                                out_ap=st[:, i, :],
                                data_ap=val_t[:, kt2, i, h, :],
                                idxs_ap=idx_t[:, kt2, i, h, :],
                                channels=128, num_elems=512, num_idxs=NIDX,
                            )
                        S_tiles[(kt2, h)] = st

                # --- input DMAs (SP queue, transfer order = issue order).
                # EA/e all load first: every mul is ready before the first
                # sub, so muls never block the DVE sub stream; q/r halves
                # stream after and the subs chase them. ---
                ea_ts, e_ts, q_ts, r_ts = [], [], {}, {}

                def load_ea(it):
                    sl = slice(it * BT, (it + 1) * BT)
                    ea_t = io.tile([128, NE], f32, tag="ea", bufs=3,
                                   name=f"ea{it}_{_rep}")
                    e_t = io.tile([128, NE], f32, tag="e", bufs=3,
                                  name=f"e{it}_{_rep}")
                    nc.sync.dma_start(out=ea_t, in_=EA[sl, :])
                    nc.sync.dma_start(out=e_t, in_=ee[sl, :])
                    ea_ts.append(ea_t)
                    e_ts.append(e_t)

                def load_qr(it, h):
                    sl = slice(it * BT, (it + 1) * BT)
                    cs = slice(h * HW2, (h + 1) * HW2)
                    q_t = io.tile([128, HW2], f32, tag="q", bufs=8,
                                  name=f"q{it}_{h}_{_rep}")
                    r_t = io.tile([128, HW2], f32, tag="r", bufs=8,
                                  name=f"r{it}_{h}_{_rep}")
                    nc.sync.dma_start(out=q_t, in_=qq[sl, cs])
                    nc.sync.dma_start(out=r_t, in_=rr[sl, cs])
                    q_ts[(it, h)] = q_t
                    r_ts[(it, h)] = r_t

                def load_qr_split(it, h):
                    # Final tile: 512-wide quarters, r halves last, so the
                    # terminal sub/square chain is as short as possible.
                    sl = slice(it * BT, (it + 1) * BT)
                    tiles = {}
                    for nm, cb2 in (("q", 0), ("q", 1), ("r", 0)):
                        t = io.tile([128, 512], f32, tag=f"{nm}s",
                                    bufs=2, name=f"{nm}{it}_{h}_{cb2}_{_rep}")
                        tiles[(nm, cb2)] = t
                    ra = io.tile([128, 256], f32, tag="rq", bufs=2,
                                 name=f"ra{it}_{h}_{_rep}")
                    rb = io.tile([128, 256], f32, tag="rq", bufs=2,
                                 name=f"rb{it}_{h}_{_rep}")
                    nc.sync.dma_start(out=tiles[("q", 0)], in_=qq[sl, slice(h * HW2, h * HW2 + 512)])
                    nc.sync.dma_start(out=tiles[("q", 1)], in_=qq[sl, slice(h * HW2 + 512, h * HW2 + 1024)])
                    nc.sync.dma_start(out=tiles[("r", 0)], in_=rr[sl, slice(h * HW2, h * HW2 + 512)])
                    nc.sync.dma_start(out=ra, in_=rr[sl, slice(h * HW2 + 512, h * HW2 + 768)])
                    nc.sync.dma_start(out=rb, in_=rr[sl, slice(h * HW2 + 768, h * HW2 + 1024)])
                    tiles[("r", 1)] = (ra, rb)
                    qr_split[(it, h)] = tiles

                # q/r arrive in it-order 2,3,0,1: the slow Pool sub/square
                # chains take the EARLY its (overlapped with the stream),
                # the fast DVE+Act chains take the tail.
                QR_ORDER = (0, 1, 2, 3)
                _PSUB = {(2, 3), (3, 1)}
                _VSQ = set()
                LAST_IT = QR_ORDER[-1]
                qr_split = {}
                for it in range(NT):
                    load_ea(it)
                for it in QR_ORDER:
                    load_qr(it, 0)
                    if it == LAST_IT:
                        load_qr_split(it, 1)
                    else:
                        load_qr(it, 1)

                # --- ax = EA*e (fp8, DVE), then 16 PE chunk-transposes into
                # one fp8 PSUM bank (start only on chunk 0 so the bank's
                # zero-region survives), Act evacuates to SBUF. axT[p, f] with
                # f = kt*128 + b holds ax[b, kt*128 + p]: sigma(e) = kt*128+p.
                axTs = []

                def make_axT(it):
                    ax = work.tile([128, NE], mybir.dt.bfloat16, tag="ax",
                                   bufs=3, name=f"ax{it}_{_rep}")
                    nc.vector.tensor_mul(ax, ea_ts[it], e_ts[it])
                    psT = psp.tile([128, NE], mybir.dt.bfloat16, tag="pst",
                                   bufs=2, name=f"psT{it}_{_rep}")
                    for c in range(NE // 128):
                        nc.tensor.matmul(
                            psT[:, c * 128:(c + 1) * 128],
                            lhsT=ax[:, c * 128:(c + 1) * 128],
                            rhs=id_t,
                            start=(c == 0), stop=(c == NE // 128 - 1),
                            is_transpose=True,
                        )
                    axT = sconst.tile([128, NE], fp8, name=f"axT{it}_{_rep}")
                    nc.scalar.copy(axT, psT)
                    axTs.append(axT)

                for it in range(NT):
                    make_axT(it)

                acc = sconst.tile([128, NCOL], f32)
                d_ts = {}

                def mm_group(it, cb):
                    # 8 DoubleRow matmuls accumulating out[b, cols] over e,
                    # then Act copies PSUM->SBUF to free the bank.
                    h, cb2 = divmod(cb, 2)
                    ps = psp.tile([128, 512], mybir.dt.float32, tag="ps",
                                  bufs=4, name=f"ps_{it}_{cb}_{_rep}")
                    for kt2 in range(KT2):
                        lhsT = axTs[it][:, 256 * kt2:256 * kt2 + 256].rearrange(
                            "p (i b) -> p i b", i=2)
                        rhs = S_tiles[(kt2, h)][:, :, :].bitcast(
                            fp8)[:, :, cb2 * 512:(cb2 + 1) * 512]
                        nc.tensor.matmul(ps, lhsT=lhsT, rhs=rhs,
                                         start=(kt2 == 0), stop=(kt2 == KT2 - 1),
                                         perf_mode=DR)
                    d = work.tile([128, 512], mybir.dt.float32, tag="d",
                                  bufs=12, name=f"d_{it}_{cb}_{_rep}")
                    nc.scalar.copy(d, ps)
                    d_ts[(it, cb)] = d

                def _qr_refs(it, cb):
                    h, cb2 = divmod(cb, 2)
                    if (it, h) in qr_split:
                        return (qr_split[(it, h)][("q", cb2)],
                                qr_split[(it, h)][("r", cb2)])
                    cs = slice(cb2 * 512, (cb2 + 1) * 512)
                    return q_ts[(it, h)][:, cs], r_ts[(it, h)][:, cs]

                def _sub_eng(it, cb):
                    return nc.gpsimd if (it, cb) in _PSUB else nc.vector

                def sub_block_q(it, cb):
                    d = d_ts[(it, cb)]
                    _sub_eng(it, cb).tensor_sub(d, d, _qr_refs(it, cb)[0])

                def sub_block_r(it, cb):
                    d = d_ts[(it, cb)]
                    _sub_eng(it, cb).tensor_sub(d, d, _qr_refs(it, cb)[1])

                def sub_block(it, cb):
                    sub_block_q(it, cb)
                    sub_block_r(it, cb)

                def square_block(it, cb):
                    # Pool-subbed blocks square on Pool (stt); the terminal
                    # block on DVE (stt); the rest on Act — three parallel
                    # tail chains.
                    col = it * NBLK + cb
                    d = d_ts[(it, cb)]
                    if (it, cb) in _VSQ or (it, cb) == (LAST_IT, NBLK - 1):
                        eng = nc.vector
                        junk = work.tile([128, 512], mybir.dt.float32,
                                         tag="jqv", bufs=2,
                                         name=f"jqv_{it}_{cb}_{_rep}")
                        eng.scalar_tensor_tensor(
                            out=junk, in0=d, scalar=1.0, in1=d,
                            op0=mybir.AluOpType.mult, op1=mybir.AluOpType.mult,
                            accum_out=acc[:, col:col + 1])
                    else:
                        junk = work.tile([128, 512], mybir.dt.bfloat16,
                                         tag="jq", bufs=2,
                                         name=f"jq_{it}_{cb}_{_rep}")
                        nc.scalar.activation(junk, d, Square,
                                             accum_out=acc[:, col:col + 1])

                # PE: column-half 0 groups chase the S build (per-kt2 chunks
                # land progressively), then half 1 once the build completes.
                # Act copies free each PSUM bank right after its group closes.
                for it in range(NT):
                    mm_group(it, 0)
                    mm_group(it, 1)
                for it in range(NT):
                    mm_group(it, 2)
                    mm_group(it, 3)
                # subs + squares stream in q/r arrival order; the last it is
                # software-pipelined (all q-subs, then r-subs, then squares)
                # so no engine queue blocks on a later load.
                for it in QR_ORDER[:-1]:
                    for cb in range(NBLK):
                        sub_block(it, cb)
                        square_block(it, cb)
                it = LAST_IT
                for cb in range(NBLK):
                    sub_block_q(it, cb)
                for cb in range(NBLK - 1):
                    sub_block_r(it, cb)
                    square_block(it, cb)
                # terminal block: two 256-wide chains, separate acc columns
                d = d_ts[(it, NBLK - 1)]
                ra, rb = qr_split[(it, 1)][("r", 1)]
                for half, rr_t in ((0, ra), (1, rb)):
                    dh = d[:, half * 256:(half + 1) * 256]
                    nc.vector.tensor_sub(dh, dh, rr_t)
                    junk = work.tile([128, 256], mybir.dt.float32,
                                     tag="jqt", bufs=2,
                                     name=f"jqt_{half}_{_rep}")
                    col = it * NBLK + NBLK - 1 + half
                    nc.vector.scalar_tensor_tensor(
                        out=junk, in0=dh, scalar=1.0, in1=dh,
                        op0=mybir.AluOpType.mult, op1=mybir.AluOpType.mult,
                        accum_out=acc[:, col:col + 1])

            nc.sync.dma_start(out=out[:, :NCOL], in_=acc)

    nc.compile()
    return nc


def _get_bass():
    if "nc" not in _CACHE:
        _CACHE["nc"] = _build_bass()
    return _CACHE["nc"]


def _build_tables(vecs, node_ids, elem_ids):
    """Compact (idx, fp8-pair) scatter tables.

    Row mapping sigma: e = 256*kt2 + 128*i + p (kt = e//128, p = e%128);
    col pair (2n, 2n+1) packs into int16 slot n = h*512 + loc
    (little-endian: low byte = col 2n).
    """
    f8 = ml_dtypes.float8_e4m3
    buckets = {}
    for k in range(E2):
        e_row = int(elem_ids[k])
        n = int(node_ids[k])
        key = (e_row, n)
        v = buckets.get(key)
        if v is None:
            buckets[key] = [float(vecs[k, 0]), float(vecs[k, 1])]
        else:
            v[0] += float(vecs[k, 0])
            v[1] += float(vecs[k, 1])
    sidx = np.full((128, KT2, 2, 2, NIDX), -1, dtype=np.int16)
    sval = np.zeros((128, KT2, 2, 2, NIDX), dtype=np.int16)
    fill = np.zeros((128, KT2, 2, 2), dtype=np.int32)
    for (e_row, n), (v0, v1) in buckets.items():
        kt, p = divmod(e_row, 128)
        kt2, i = divmod(kt, 2)
        h, loc = divmod(n, 512)
        j = fill[p, kt2, i, h]
        assert j < NIDX, f"bucket overflow at {(p, kt2, i, h)}"
        fill[p, kt2, i, h] = j + 1
        lo = np.float32(v0).astype(f8).view(np.uint8)
        hi = np.float32(v1).astype(f8).view(np.uint8)
        sidx[p, kt2, i, h, j] = loc
        sval[p, kt2, i, h, j] = np.int16(
            np.uint16(int(lo) | (int(hi) << 8)).view(np.int16))
    return sidx, sval


def _prep_in_maps(EA, e, q, r, vecs, node_ids, elem_ids):
    EA = np.ascontiguousarray(np.asarray(EA, dtype=np.float32))
    e = np.ascontiguousarray(np.asarray(e, dtype=np.float32))
    q = np.ascontiguousarray(np.asarray(q, dtype=np.float32)).reshape(B, N2)
    r = np.ascontiguousarray(np.asarray(r, dtype=np.float32)).reshape(B, N2)
    vecs = np.asarray(vecs, dtype=np.float32)
    sidx, sval = _build_tables(vecs, np.asarray(node_ids), np.asarray(elem_ids))
    ident = np.eye(128, dtype=np.float32).astype(ml_dtypes.bfloat16)
    tabs = np.concatenate([
        sidx.reshape(128, 512),
        sval.reshape(128, 512),
        ident.view(np.int16),
    ], axis=1).astype(np.int16)

    in_maps = []
    for c in range(NCORES):
        sl = slice(c * SHARD, (c + 1) * SHARD)
        in_maps.append({
            "EA": EA[sl], "e": e[sl], "q": q[sl], "r": r[sl],
            "tabs": tabs,
        })
    return in_maps


def _reduce_outs(results):
    total = 0.0
    for c in range(NCORES):
        total += results[c]["out"][:, :NT * NBLK + 1].astype(np.float64).sum()
    return np.array(total / (B * NN * 2), dtype=np.float32)


def kernel_run(EA, e, q, r, vecs, node_ids, elem_ids, trace=False):
    from concourse.bass_utils import run_bass_kernel_spmd

    nc = _get_bass()
    in_maps = _prep_in_maps(EA, e, q, r, vecs, node_ids, elem_ids)
    res = run_bass_kernel_spmd(nc, in_maps, core_ids=list(range(NCORES)),
                               trace=trace)
    return _reduce_outs(res.results), res


def kernel(EA, e, q, r, vecs, node_ids, elem_ids):
    val, _ = kernel_run(EA, e, q, r, vecs, node_ids, elem_ids, trace=False)
    return val


# revision 57
# speedup vs baseline: 1.0031x; 1.0031x over previous
"""NodeEquilibriumLoss Trainium2 kernel (fp8 DoubleRow version).

residual[b] = (EA[b] * e[b]) @ S - q[b] - r[b];  out = mean(residual^2)

S[elem, 2*node+c] = sum_k [elem_ids[k]==elem][node_ids[k]==node] * vecs[k, c]
is the fixed sparse linear map implementing the reference's gather+scatter-add.

Sharding: data-parallel over batch, 8 cores x 512 rows. Per core:
  - S is held in fp8e4 with row mapping sigma(e) = kt*128 + p, packed as
    int16 pairs (cols 2n/2n+1 share one int16 slot) and built on device from
    compact (index, packed-value) tables via 32 gpsimd.local_scatter calls
    (~26us on Pool, fully overlapped with the input DMA stream). One tile
    per (kt2, h) window keeps the matmul rhs dependencies call-precise.
  - per 128-row batch tile: ax = EA*e in bf16 (DVE), 16 PE chunk-transposes
    into one bf16 PSUM bank (start only on chunk 0 so the bank zero-region
    survives), Act evacuates to SBUF with a cast to fp8 -> axT[p, kt*128+b].
  - 8 fp8 DoubleRow matmuls (K=256 each: lhsT dim1 = adjacent kt pair)
    accumulate each [128, 512] output block in PSUM; a copy frees the bank
    (Act for early tiles, DVE late), then d -= q, d -= r and a
    square+accumulate into acc[:, col] stream behind the q/r loads, spread
    across DVE/Pool/Act as three parallel tail chains (the terminal block's
    chain is the shortest: DVE sub + DVE square).
  - DMA order: one table blob (scatter idx/val + transpose identity) first
    on the SP queue, then EA/e, then q/r halves; the last tile's q/r arrive
    as 512/256-col quarters and the terminal block squares as two 256-wide
    DVE chains into separate acc columns, so the post-last-load chain is
    minimal. All loads are chain-ready when scheduled, keeping the global
    DMA stream dense (~47.5us, the memory roofline).
  - output: [128, 17] per-partition partial sums; host reduces in fp64.
"""

import numpy as np
import ml_dtypes

B, NE, NN, E2 = 4096, 2048, 1024, 4096
N2 = 2 * NN
NCORES = 8
SHARD = B // NCORES   # 512
BT = 128              # batch rows per tile
NT = SHARD // BT      # 4 batch tiles per core
KT2 = 8               # DoubleRow contraction tiles (K=256 each)
NBLK = 4              # output column blocks of 512 per batch tile
NIDX = 8              # padded nonzeros per (elem row, col half); measured max 6

_CACHE = {}


def _build_bass(reps=1, out_pad=0):
    # reps>1 repeats the whole computation (idempotently) inside one NEFF;
    # out_pad widens the output tensor so the HLO (and the libneuronxla NEFF
    # cache key, which ignores the embedded BIR) differs between variants.
    # Both are used only by timing harnesses.
    from concourse import bacc
    import concourse.mybir as mybir
    import concourse.tile as tile

    f32 = mybir.dt.float32
    fp8 = mybir.dt.float8e4
    i16 = mybir.dt.int16
    DR = mybir.MatmulPerfMode.DoubleRow
    Square = mybir.ActivationFunctionType.Square

    nc = bacc.Bacc("TRN2", target_bir_lowering=False, debug=False,
                   num_devices=NCORES)
    EA = nc.dram_tensor("EA", [SHARD, NE], f32, kind="ExternalInput").ap()
    ee = nc.dram_tensor("e", [SHARD, NE], f32, kind="ExternalInput").ap()
    qq = nc.dram_tensor("q", [SHARD, N2], f32, kind="ExternalInput").ap()
    rr = nc.dram_tensor("r", [SHARD, N2], f32, kind="ExternalInput").ap()
    # one blob per partition: sidx [256 i16] | sval [256 i16] | ident
    # [128 bf16 = 128 i16] -> 640 i16; a single DMA feeds the scatter
    # chain + transposes with minimal head latency
    tabs = nc.dram_tensor("tabs", [128, 640], i16,
                          kind="ExternalInput").ap()
    NCOL = NT * NBLK + 1  # terminal block contributes two half-columns
    out = nc.dram_tensor("out", [128, NCOL + out_pad], f32,
                         kind="ExternalOutput").ap()

    HW2 = N2 // 2   # 1024 cols per q/r half-load

    with tile.TileContext(nc) as tc:
        with (
            tc.tile_pool(name="sconst", bufs=1) as sconst,
            tc.tile_pool(name="io", bufs=2) as io,
            tc.tile_pool(name="work", bufs=2) as work,
            tc.tile_pool(name="ps", bufs=8, space="PSUM") as psp,
        ):
            for _rep in range(reps):
                # --- tables + S build (Pool; overlaps everything) ---
                tab_t = sconst.tile([128, 640], i16)
                # On the SP queue (fastest DMA issue path) ahead of EA/e so
                # the scatter chain starts immediately.
                with tc.high_priority():
                    nc.sync.dma_start(out=tab_t, in_=tabs)
                idx_t = tab_t[:, 0:256].rearrange(
                    "p (a b c d) -> p a b c d", a=KT2, b=2, c=2, d=NIDX)
                val_t = tab_t[:, 256:512].rearrange(
                    "p (a b c d) -> p a b c d", a=KT2, b=2, c=2, d=NIDX)
                id_t = tab_t[:, 512:640].bitcast(mybir.dt.bfloat16)
                # One tile per (kt2, h) window [p, i, slot] so the matmul's
                # rhs dependency is exactly that window's two scatter calls
                # (cross-dtype subtile range tracking is imprecise).
                S_tiles = {}
                for h in range(2):
                    for kt2 in range(KT2):
                        st = sconst.tile([128, 2, 512], i16,
                                         name=f"S_{kt2}_{h}_{_rep}")
                        for i in range(2):
                            nc.gpsimd.local_scatter(
                                out_ap=st[:, i, :],
                                data_ap=val_t[:, kt2, i, h, :],
                                idxs_ap=idx_t[:, kt2, i, h, :],
                                channels=128, num_elems=512, num_idxs=NIDX,
                            )
                        S_tiles[(kt2, h)] = st

                # --- input DMAs (SP queue, transfer order = issue order).
                # EA/e all load first: every mul is ready before the first
                # sub, so muls never block the DVE sub stream; q/r halves
                # stream after and the subs chase them. ---
                ea_ts, e_ts, q_ts, r_ts = [], [], {}, {}

                def load_ea(it):
                    sl = slice(it * BT, (it + 1) * BT)
                    ea_t = io.tile([128, NE], f32, tag="ea", bufs=3,
                                   name=f"ea{it}_{_rep}")
                    e_t = io.tile([128, NE], f32, tag="e", bufs=3,
                                  name=f"e{it}_{_rep}")
                    nc.sync.dma_start(out=ea_t, in_=EA[sl, :])
                    nc.sync.dma_start(out=e_t, in_=ee[sl, :])
                    ea_ts.append(ea_t)
                    e_ts.append(e_t)

                def load_qr(it, h):
                    sl = slice(it * BT, (it + 1) * BT)
                    cs = slice(h * HW2, (h + 1) * HW2)
                    q_t = io.tile([128, HW2], f32, tag="q", bufs=8,
                                  name=f"q{it}_{h}_{_rep}")
                    r_t = io.tile([128, HW2], f32, tag="r", bufs=8,
                                  name=f"r{it}_{h}_{_rep}")
                    nc.sync.dma_start(out=q_t, in_=qq[sl, cs])
                    nc.sync.dma_start(out=r_t, in_=rr[sl, cs])
                    q_ts[(it, h)] = q_t
                    r_ts[(it, h)] = r_t

                def load_qr_split(it, h):
                    # Final tile: 512-wide quarters, r halves last, so the
                    # terminal sub/square chain is as short as possible.
                    sl = slice(it * BT, (it + 1) * BT)
                    tiles = {}
                    for nm, cb2 in (("q", 0), ("q", 1), ("r", 0)):
                        t = io.tile([128, 512], f32, tag=f"{nm}s",
                                    bufs=2, name=f"{nm}{it}_{h}_{cb2}_{_rep}")
                        tiles[(nm, cb2)] = t
                    ra = io.tile([128, 256], f32, tag="rq", bufs=2,
                                 name=f"ra{it}_{h}_{_rep}")
                    rb = io.tile([128, 256], f32, tag="rq", bufs=2,
                                 name=f"rb{it}_{h}_{_rep}")
                    nc.sync.dma_start(out=tiles[("q", 0)], in_=qq[sl, slice(h * HW2, h * HW2 + 512)])
                    nc.sync.dma_start(out=tiles[("q", 1)], in_=qq[sl, slice(h * HW2 + 512, h * HW2 + 1024)])
                    nc.sync.dma_start(out=tiles[("r", 0)], in_=rr[sl, slice(h * HW2, h * HW2 + 512)])
                    nc.sync.dma_start(out=ra, in_=rr[sl, slice(h * HW2 + 512, h * HW2 + 768)])
                    nc.sync.dma_start(out=rb, in_=rr[sl, slice(h * HW2 + 768, h * HW2 + 1024)])
                    tiles[("r", 1)] = (ra, rb)
                    qr_split[(it, h)] = tiles

                # q/r arrive in it-order 2,3,0,1: the slow Pool sub/square
                # chains take the EARLY its (overlapped with the stream),
                # the fast DVE+Act chains take the tail.
                QR_ORDER = (0, 1, 2, 3)
                _PSUB = {(2, 3), (3, 1)}
                _VSQ = set()
                LAST_IT = QR_ORDER[-1]
                qr_split = {}
                for it in range(NT):
                    load_ea(it)
                for it in QR_ORDER:
                    load_qr(it, 0)
                    if it == LAST_IT:
                        load_qr_split(it, 1)
                    else:
                        load_qr(it, 1)

                # --- ax = EA*e (fp8, DVE), then 16 PE chunk-transposes into
                # one fp8 PSUM bank (start only on chunk 0 so the bank's
                # zero-region survives), Act evacuates to SBUF. axT[p, f] with
                # f = kt*128 + b holds ax[b, kt*128 + p]: sigma(e) = kt*128+p.
                axTs = []

                def make_axT(it):
                    ax = work.tile([128, NE], mybir.dt.bfloat16, tag="ax",
                                   bufs=3, name=f"ax{it}_{_rep}")
                    nc.vector.tensor_mul(ax, ea_ts[it], e_ts[it])
                    psT = psp.tile([128, NE], mybir.dt.bfloat16, tag="pst",
                                   bufs=2, name=f"psT{it}_{_rep}")
                    for c in range(NE // 128):
                        nc.tensor.matmul(
                            psT[:, c * 128:(c + 1) * 128],
                            lhsT=ax[:, c * 128:(c + 1) * 128],
                            rhs=id_t,
                            start=(c == 0), stop=(c == NE // 128 - 1),
                            is_transpose=True,
                        )
                    axT = sconst.tile([128, NE], fp8, name=f"axT{it}_{_rep}")
                    nc.scalar.copy(axT, psT)
                    axTs.append(axT)

                for it in range(NT):
                    make_axT(it)

                acc = sconst.tile([128, NCOL], f32)
                d_ts = {}

                def mm_group(it, cb):
                    # 8 DoubleRow matmuls accumulating out[b, cols] over e,
                    # then Act copies PSUM->SBUF to free the bank.
                    h, cb2 = divmod(cb, 2)
                    ps = psp.tile([128, 512], mybir.dt.float32, tag="ps",
                                  bufs=4, name=f"ps_{it}_{cb}_{_rep}")
                    for kt2 in range(KT2):
                        lhsT = axTs[it][:, 256 * kt2:256 * kt2 + 256].rearrange(
                            "p (i b) -> p i b", i=2)
                        rhs = S_tiles[(kt2, h)][:, :, :].bitcast(
                            fp8)[:, :, cb2 * 512:(cb2 + 1) * 512]
                        nc.tensor.matmul(ps, lhsT=lhsT, rhs=rhs,
                                         start=(kt2 == 0), stop=(kt2 == KT2 - 1),
                                         perf_mode=DR)
                    d = work.tile([128, 512], mybir.dt.float32, tag="d",
                                  bufs=12, name=f"d_{it}_{cb}_{_rep}")
                    nc.scalar.copy(d, ps)
                    d_ts[(it, cb)] = d

                def _qr_refs(it, cb):
                    h, cb2 = divmod(cb, 2)
                    if (it, h) in qr_split:
                        return (qr_split[(it, h)][("q", cb2)],
                                qr_split[(it, h)][("r", cb2)])
                    cs = slice(cb2 * 512, (cb2 + 1) * 512)
                    return q_ts[(it, h)][:, cs], r_ts[(it, h)][:, cs]

                def _sub_eng(it, cb):
                    return nc.gpsimd if (it, cb) in _PSUB else nc.vector

                def sub_block_q(it, cb):
                    d = d_ts[(it, cb)]
                    _sub_eng(it, cb).tensor_sub(d, d, _qr_refs(it, cb)[0])

                def sub_block_r(it, cb):
                    d = d_ts[(it, cb)]
                    _sub_eng(it, cb).tensor_sub(d, d, _qr_refs(it, cb)[1])

                def sub_block(it, cb):
                    sub_block_q(it, cb)
                    sub_block_r(it, cb)

                def square_block(it, cb):
                    # Pool-subbed blocks square on Pool (stt); the terminal
                    # block on DVE (stt); the rest on Act — three parallel
                    # tail chains.
                    col = it * NBLK + cb
                    d = d_ts[(it, cb)]
                    if (it, cb) in _VSQ or (it, cb) == (LAST_IT, NBLK - 1):
                        eng = nc.vector
                        junk = work.tile([128, 512], mybir.dt.float32,
                                         tag="jqv", bufs=2,
                                         name=f"jqv_{it}_{cb}_{_rep}")
                        eng.scalar_tensor_tensor(
                            out=junk, in0=d, scalar=1.0, in1=d,
                            op0=mybir.AluOpType.mult, op1=mybir.AluOpType.mult,
                            accum_out=acc[:, col:col + 1])
                    else:
                        junk = work.tile([128, 512], mybir.dt.bfloat16,
                                         tag="jq", bufs=2,
                                         name=f"jq_{it}_{cb}_{_rep}")
                        nc.scalar.activation(junk, d, Square,
                                             accum_out=acc[:, col:col + 1])

                # PE: column-half 0 groups chase the S build (per-kt2 chunks
                # land progressively), then half 1 once the build completes.
                # Act copies free each PSUM bank right after its group closes.
                for it in range(NT):
                    mm_group(it, 0)
                    mm_group(it, 1)
                for it in range(NT):
                    mm_group(it, 2)
                    mm_group(it, 3)
                # subs + squares stream in q/r arrival order; the last it is
                # software-pipelined (all q-subs, then r-subs, then squares)
                # so no engine queue blocks on a later load.
                for it in QR_ORDER[:-1]:
                    for cb in range(NBLK):
                        sub_block(it, cb)
                        square_block(it, cb)
                it = LAST_IT
                for cb in range(NBLK):
                    sub_block_q(it, cb)
                for cb in range(NBLK - 1):
                    sub_block_r(it, cb)
                    square_block(it, cb)
                # terminal block: two 256-wide chains, separate acc columns
                d = d_ts[(it, NBLK - 1)]
                ra, rb = qr_split[(it, 1)][("r", 1)]
                for half, rr_t in ((0, ra), (1, rb)):
                    dh = d[:, half * 256:(half + 1) * 256]
                    nc.vector.tensor_sub(dh, dh, rr_t)
                    junk = work.tile([128, 256], mybir.dt.float32,
                                     tag="jqt", bufs=2,
                                     name=f"jqt_{half}_{_rep}")
                    col = it * NBLK + NBLK - 1 + half
                    nc.vector.scalar_tensor_tensor(
                        out=junk, in0=dh, scalar=1.0, in1=dh,
                        op0=mybir.AluOpType.mult, op1=mybir.AluOpType.mult,
                        accum_out=acc[:, col:col + 1])

            nc.sync.dma_start(out=out[:, :NCOL], in_=acc)

    nc.compile()
    return nc


def _get_bass():
    if "nc" not in _CACHE:
        _CACHE["nc"] = _build_bass()
    return _CACHE["nc"]


def _build_tables(vecs, node_ids, elem_ids):
    """Compact (idx, fp8-pair) scatter tables.

    Row mapping sigma: e = 256*kt2 + 128*i + p (kt = e//128, p = e%128);
    col pair (2n, 2n+1) packs into int16 slot n = h*512 + loc
    (little-endian: low byte = col 2n).
    """
    f8 = ml_dtypes.float8_e4m3
    buckets = {}
    for k in range(E2):
        e_row = int(elem_ids[k])
        n = int(node_ids[k])
        key = (e_row, n)
        v = buckets.get(key)
        if v is None:
            buckets[key] = [float(vecs[k, 0]), float(vecs[k, 1])]
        else:
            v[0] += float(vecs[k, 0])
            v[1] += float(vecs[k, 1])
    sidx = np.full((128, KT2, 2, 2, NIDX), -1, dtype=np.int16)
    sval = np.zeros((128, KT2, 2, 2, NIDX), dtype=np.int16)
    fill = np.zeros((128, KT2, 2, 2), dtype=np.int32)
    for (e_row, n), (v0, v1) in buckets.items():
        kt, p = divmod(e_row, 128)
        kt2, i = divmod(kt, 2)
        h, loc = divmod(n, 512)
        j = fill[p, kt2, i, h]
        assert j < NIDX, f"bucket overflow at {(p, kt2, i, h)}"
        fill[p, kt2, i, h] = j + 1
        lo = np.float32(v0).astype(f8).view(np.uint8)
        hi = np.float32(v1).astype(f8).view(np.uint8)
        sidx[p, kt2, i, h, j] = loc
        sval[p, kt2, i, h, j] = np.int16(
            np.uint16(int(lo) | (int(hi) << 8)).view(np.int16))
    return sidx, sval


def _prep_in_maps(EA, e, q, r, vecs, node_ids, elem_ids):
    EA = np.ascontiguousarray(np.asarray(EA, dtype=np.float32))
    e = np.ascontiguousarray(np.asarray(e, dtype=np.float32))
    q = np.ascontiguousarray(np.asarray(q, dtype=np.float32)).reshape(B, N2)
    r = np.ascontiguousarray(np.asarray(r, dtype=np.float32)).reshape(B, N2)
    vecs = np.asarray(vecs, dtype=np.float32)
    sidx, sval = _build_tables(vecs, np.asarray(node_ids), np.asarray(elem_ids))
    ident = np.eye(128, dtype=np.float32).astype(ml_dtypes.bfloat16)
    tabs = np.concatenate([
        sidx.reshape(128, 256),
        sval.reshape(128, 256),
        ident.view(np.int16),
    ], axis=1).astype(np.int16)

    in_maps = []
    for c in range(NCORES):
        sl = slice(c * SHARD, (c + 1) * SHARD)
        in_maps.append({
            "EA": EA[sl], "e": e[sl], "q": q[sl], "r": r[sl],
            "tabs": tabs,
        })
    return in_maps


def _reduce_outs(results):
    total = 0.0
    for c in range(NCORES):
        total += results[c]["out"][:, :NT * NBLK + 1].astype(np.float64).sum()
    return np.array(total / (B * NN * 2), dtype=np.float32)


def kernel_run(EA, e, q, r, vecs, node_ids, elem_ids, trace=False):
    from concourse.bass_utils import run_bass_kernel_spmd

    nc = _get_bass()
    in_maps = _prep_in_maps(EA, e, q, r, vecs, node_ids, elem_ids)
    res = run_bass_kernel_spmd(nc, in_maps, core_ids=list(range(NCORES)),
                               trace=trace)
    return _reduce_outs(res.results), res


def kernel(EA, e, q, r, vecs, node_ids, elem_ids):
    val, _ = kernel_run(EA, e, q, r, vecs, node_ids, elem_ids, trace=False)
    return val


# revision 61
# speedup vs baseline: 1.0038x; 1.0007x over previous
"""NodeEquilibriumLoss Trainium2 kernel (fp8 DoubleRow version).

residual[b] = (EA[b] * e[b]) @ S - q[b] - r[b];  out = mean(residual^2)

S[elem, 2*node+c] = sum_k [elem_ids[k]==elem][node_ids[k]==node] * vecs[k, c]
is the fixed sparse linear map implementing the reference's gather+scatter-add.

Sharding: data-parallel over batch, 8 cores x 512 rows. Per core:
  - S is held in fp8e4 with row mapping sigma(e) = kt*128 + p, packed as
    int16 pairs (cols 2n/2n+1 share one int16 slot) and built on device from
    compact (index, packed-value) tables via 32 gpsimd.local_scatter calls
    (~26us on Pool, fully overlapped with the input DMA stream). One tile
    per (kt2, h) window keeps the matmul rhs dependencies call-precise.
  - per 128-row batch tile: ax = EA*e in bf16 (DVE), 16 PE chunk-transposes
    into one bf16 PSUM bank (start only on chunk 0 so the bank zero-region
    survives), Act evacuates to SBUF with a cast to fp8 -> axT[p, kt*128+b].
  - 8 fp8 DoubleRow matmuls (K=256 each: lhsT dim1 = adjacent kt pair)
    accumulate each [128, 512] output block in PSUM; a copy frees the bank
    (Act for early tiles, DVE late), then d -= q, d -= r and a
    square+accumulate into acc[:, col] stream behind the q/r loads, spread
    across DVE/Pool/Act as three parallel tail chains (the terminal block's
    chain is the shortest: DVE sub + DVE square).
  - DMA order: one table blob (scatter idx/val + transpose identity) first
    on the SP queue, then EA/e, then q/r halves; the last tile's q/r arrive
    as 512/256-col quarters and the terminal block squares as two 256-wide
    DVE chains into separate acc columns, so the post-last-load chain is
    minimal. All loads are chain-ready when scheduled, keeping the global
    DMA stream dense (~47.5us, the memory roofline).
  - output: [128, 17] per-partition partial sums; host reduces in fp64.
"""

import numpy as np
import ml_dtypes

B, NE, NN, E2 = 4096, 2048, 1024, 4096
N2 = 2 * NN
NCORES = 8
SHARD = B // NCORES   # 512
BT = 128              # batch rows per tile
NT = SHARD // BT      # 4 batch tiles per core
KT2 = 8               # DoubleRow contraction tiles (K=256 each)
NBLK = 4              # output column blocks of 512 per batch tile
NIDX = 8              # padded nonzeros per (elem row, col half); measured max 6

_CACHE = {}


def _build_bass(reps=1, out_pad=0):
    # reps>1 repeats the whole computation (idempotently) inside one NEFF;
    # out_pad widens the output tensor so the HLO (and the libneuronxla NEFF
    # cache key, which ignores the embedded BIR) differs between variants.
    # Both are used only by timing harnesses.
    from concourse import bacc
    import concourse.mybir as mybir
    import concourse.tile as tile

    f32 = mybir.dt.float32
    fp8 = mybir.dt.float8e4
    i16 = mybir.dt.int16
    DR = mybir.MatmulPerfMode.DoubleRow
    Square = mybir.ActivationFunctionType.Square

    nc = bacc.Bacc("TRN2", target_bir_lowering=False, debug=False,
                   num_devices=NCORES)
    EA = nc.dram_tensor("EA", [SHARD, NE], f32, kind="ExternalInput").ap()
    ee = nc.dram_tensor("e", [SHARD, NE], f32, kind="ExternalInput").ap()
    qq = nc.dram_tensor("q", [SHARD, N2], f32, kind="ExternalInput").ap()
    rr = nc.dram_tensor("r", [SHARD, N2], f32, kind="ExternalInput").ap()
    # one blob per partition: sidx [256 i16] | sval [256 i16] | ident
    # [128 bf16 = 128 i16] -> 640 i16; a single DMA feeds the scatter
    # chain + transposes with minimal head latency
    tabs = nc.dram_tensor("tabs", [128, 640], i16,
                          kind="ExternalInput").ap()
    NCOL = NT * NBLK + 1  # terminal block contributes two half-columns
    out = nc.dram_tensor("out", [128, NCOL + out_pad], f32,
                         kind="ExternalOutput").ap()

    HW2 = N2 // 2   # 1024 cols per q/r half-load

    with tile.TileContext(nc) as tc:
        with (
            tc.tile_pool(name="sconst", bufs=1) as sconst,
            tc.tile_pool(name="io", bufs=2) as io,
            tc.tile_pool(name="work", bufs=2) as work,
            tc.tile_pool(name="ps", bufs=8, space="PSUM") as psp,
        ):
            for _rep in range(reps):
                # --- tables + S build (Pool; overlaps everything) ---
                tab_t = sconst.tile([128, 640], i16)
                # On the SP queue (fastest DMA issue path) ahead of EA/e so
                # the scatter chain starts immediately.
                with tc.high_priority():
                    nc.sync.dma_start(out=tab_t, in_=tabs)
                idx_t = tab_t[:, 0:256].rearrange(
                    "p (a b c d) -> p a b c d", a=KT2, b=2, c=2, d=NIDX)
                val_t = tab_t[:, 256:512].rearrange(
                    "p (a b c d) -> p a b c d", a=KT2, b=2, c=2, d=NIDX)
                id_t = tab_t[:, 512:640].bitcast(mybir.dt.bfloat16)
                # One tile per (kt2, h) window [p, i, slot] so the matmul's
                # rhs dependency is exactly that window's two scatter calls
                # (cross-dtype subtile range tracking is imprecise).
                S_tiles = {}
                for h in range(2):
                    for kt2 in range(KT2):
                        st = sconst.tile([128, 2, 512], i16,
                                         name=f"S_{kt2}_{h}_{_rep}")
                        for i in range(2):
                            nc.gpsimd.local_scatter(
                                out_ap=st[:, i, :],
                                data_ap=val_t[:, kt2, i, h, :],
                                idxs_ap=idx_t[:, kt2, i, h, :],
                                channels=128, num_elems=512, num_idxs=NIDX,
                            )
                        S_tiles[(kt2, h)] = st

                # --- input DMAs (SP queue, transfer order = issue order).
                # EA/e all load first: every mul is ready before the first
                # sub, so muls never block the DVE sub stream; q/r halves
                # stream after and the subs chase them. ---
                ea_ts, e_ts, q_ts, r_ts = [], [], {}, {}

                def load_ea(it):
                    sl = slice(it * BT, (it + 1) * BT)
                    ea_t = io.tile([128, NE], f32, tag="ea", bufs=3,
                                   name=f"ea{it}_{_rep}")
                    e_t = io.tile([128, NE], f32, tag="e", bufs=3,
                                  name=f"e{it}_{_rep}")
                    nc.sync.dma_start(out=ea_t, in_=EA[sl, :])
                    nc.sync.dma_start(out=e_t, in_=ee[sl, :])
                    ea_ts.append(ea_t)
                    e_ts.append(e_t)

                def load_qr(it, h):
                    sl = slice(it * BT, (it + 1) * BT)
                    cs = slice(h * HW2, (h + 1) * HW2)
                    q_t = io.tile([128, HW2], f32, tag="q", bufs=8,
                                  name=f"q{it}_{h}_{_rep}")
                    r_t = io.tile([128, HW2], f32, tag="r", bufs=8,
                                  name=f"r{it}_{h}_{_rep}")
                    nc.sync.dma_start(out=q_t, in_=qq[sl, cs])
                    nc.sync.dma_start(out=r_t, in_=rr[sl, cs])
                    q_ts[(it, h)] = q_t
                    r_ts[(it, h)] = r_t

                def load_qr_split(it, h):
                    # Final tile: 512-wide quarters, r halves last, so the
                    # terminal sub/square chain is as short as possible.
                    sl = slice(it * BT, (it + 1) * BT)
                    tiles = {}
                    for nm, cb2 in (("q", 0), ("q", 1), ("r", 0)):
                        t = io.tile([128, 512], f32, tag=f"{nm}s",
                                    bufs=2, name=f"{nm}{it}_{h}_{cb2}_{_rep}")
                        tiles[(nm, cb2)] = t
                    ra = io.tile([128, 256], f32, tag="rq", bufs=2,
                                 name=f"ra{it}_{h}_{_rep}")
                    rb = io.tile([128, 256], f32, tag="rq", bufs=2,
                                 name=f"rb{it}_{h}_{_rep}")
                    nc.sync.dma_start(out=tiles[("q", 0)], in_=qq[sl, slice(h * HW2, h * HW2 + 512)])
                    nc.sync.dma_start(out=tiles[("q", 1)], in_=qq[sl, slice(h * HW2 + 512, h * HW2 + 1024)])
                    nc.sync.dma_start(out=tiles[("r", 0)], in_=rr[sl, slice(h * HW2, h * HW2 + 512)])
                    nc.sync.dma_start(out=ra, in_=rr[sl, slice(h * HW2 + 512, h * HW2 + 768)])
                    nc.sync.dma_start(out=rb, in_=rr[sl, slice(h * HW2 + 768, h * HW2 + 1024)])
                    tiles[("r", 1)] = (ra, rb)
                    qr_split[(it, h)] = tiles

                # q/r arrive in it-order 2,3,0,1: the slow Pool sub/square
                # chains take the EARLY its (overlapped with the stream),
                # the fast DVE+Act chains take the tail.
                QR_ORDER = (0, 1, 2, 3)
                _PSUB = {(2, 3), (3, 2)}
                _VSQ = set()
                LAST_IT = QR_ORDER[-1]
                qr_split = {}
                for it in range(NT):
                    load_ea(it)
                for it in QR_ORDER:
                    load_qr(it, 0)
                    if it == LAST_IT:
                        load_qr_split(it, 1)
                    else:
                        load_qr(it, 1)

                # --- ax = EA*e (fp8, DVE), then 16 PE chunk-transposes into
                # one fp8 PSUM bank (start only on chunk 0 so the bank's
                # zero-region survives), Act evacuates to SBUF. axT[p, f] with
                # f = kt*128 + b holds ax[b, kt*128 + p]: sigma(e) = kt*128+p.
                axTs = []

                def make_axT(it):
                    ax = work.tile([128, NE], mybir.dt.bfloat16, tag="ax",
                                   bufs=3, name=f"ax{it}_{_rep}")
                    nc.vector.tensor_mul(ax, ea_ts[it], e_ts[it])
                    psT = psp.tile([128, NE], mybir.dt.bfloat16, tag="pst",
                                   bufs=2, name=f"psT{it}_{_rep}")
                    for c in range(NE // 128):
                        nc.tensor.matmul(
                            psT[:, c * 128:(c + 1) * 128],
                            lhsT=ax[:, c * 128:(c + 1) * 128],
                            rhs=id_t,
                            start=(c == 0), stop=(c == NE // 128 - 1),
                            is_transpose=True,
                        )
                    axT = sconst.tile([128, NE], fp8, name=f"axT{it}_{_rep}")
                    nc.scalar.copy(axT, psT)
                    axTs.append(axT)

                for it in range(NT):
                    make_axT(it)

                acc = sconst.tile([128, NCOL], f32)
                d_ts = {}

                def mm_group(it, cb):
                    # 8 DoubleRow matmuls accumulating out[b, cols] over e,
                    # then Act copies PSUM->SBUF to free the bank.
                    h, cb2 = divmod(cb, 2)
                    ps = psp.tile([128, 512], mybir.dt.float32, tag="ps",
                                  bufs=4, name=f"ps_{it}_{cb}_{_rep}")
                    for kt2 in range(KT2):
                        lhsT = axTs[it][:, 256 * kt2:256 * kt2 + 256].rearrange(
                            "p (i b) -> p i b", i=2)
                        rhs = S_tiles[(kt2, h)][:, :, :].bitcast(
                            fp8)[:, :, cb2 * 512:(cb2 + 1) * 512]
                        nc.tensor.matmul(ps, lhsT=lhsT, rhs=rhs,
                                         start=(kt2 == 0), stop=(kt2 == KT2 - 1),
                                         perf_mode=DR)
                    d = work.tile([128, 512], mybir.dt.float32, tag="d",
                                  bufs=12, name=f"d_{it}_{cb}_{_rep}")
                    nc.scalar.copy(d, ps)
                    d_ts[(it, cb)] = d

                def _qr_refs(it, cb):
                    h, cb2 = divmod(cb, 2)
                    if (it, h) in qr_split:
                        return (qr_split[(it, h)][("q", cb2)],
                                qr_split[(it, h)][("r", cb2)])
                    cs = slice(cb2 * 512, (cb2 + 1) * 512)
                    return q_ts[(it, h)][:, cs], r_ts[(it, h)][:, cs]

                def _sub_eng(it, cb):
                    return nc.gpsimd if (it, cb) in _PSUB else nc.vector

                def sub_block_q(it, cb):
                    d = d_ts[(it, cb)]
                    _sub_eng(it, cb).tensor_sub(d, d, _qr_refs(it, cb)[0])

                def sub_block_r(it, cb):
                    d = d_ts[(it, cb)]
                    _sub_eng(it, cb).tensor_sub(d, d, _qr_refs(it, cb)[1])

                def sub_block(it, cb):
                    sub_block_q(it, cb)
                    sub_block_r(it, cb)

                def square_block(it, cb):
                    # Pool-subbed blocks square on Pool (stt); the terminal
                    # block on DVE (stt); the rest on Act — three parallel
                    # tail chains.
                    col = it * NBLK + cb
                    d = d_ts[(it, cb)]
                    if (it, cb) in _VSQ or (it, cb) == (LAST_IT, NBLK - 1):
                        eng = nc.vector
                        junk = work.tile([128, 512], mybir.dt.float32,
                                         tag="jqv", bufs=2,
                                         name=f"jqv_{it}_{cb}_{_rep}")
                        eng.scalar_tensor_tensor(
                            out=junk, in0=d, scalar=1.0, in1=d,
                            op0=mybir.AluOpType.mult, op1=mybir.AluOpType.mult,
                            accum_out=acc[:, col:col + 1])
                    else:
                        junk = work.tile([128, 512], mybir.dt.bfloat16,
                                         tag="jq", bufs=2,
                                         name=f"jq_{it}_{cb}_{_rep}")
                        nc.scalar.activation(junk, d, Square,
                                             accum_out=acc[:, col:col + 1])

                # PE: column-half 0 groups chase the S build (per-kt2 chunks
                # land progressively), then half 1 once the build completes.
                # Act copies free each PSUM bank right after its group closes.
                for it in range(NT):
                    mm_group(it, 0)
                    mm_group(it, 1)
                for it in range(NT):
                    mm_group(it, 2)
                    mm_group(it, 3)
                # subs + squares stream in q/r arrival order; the last it is
                # software-pipelined (all q-subs, then r-subs, then squares)
                # so no engine queue blocks on a later load.
                for it in QR_ORDER[:-1]:
                    for cb in range(NBLK):
                        sub_block(it, cb)
                        square_block(it, cb)
                it = LAST_IT
                for cb in range(NBLK):
                    sub_block_q(it, cb)
                for cb in range(NBLK - 1):
                    sub_block_r(it, cb)
                    square_block(it, cb)
                # terminal block: two 256-wide chains, separate acc columns
                d = d_ts[(it, NBLK - 1)]
                ra, rb = qr_split[(it, 1)][("r", 1)]
                for half, rr_t in ((0, ra), (1, rb)):
                    dh = d[:, half * 256:(half + 1) * 256]
                    nc.vector.tensor_sub(dh, dh, rr_t)
                    junk = work.tile([128, 256], mybir.dt.float32,
                                     tag="jqt", bufs=2,
                                     name=f"jqt_{half}_{_rep}")
                    col = it * NBLK + NBLK - 1 + half
                    nc.vector.scalar_tensor_tensor(
                        out=junk, in0=dh, scalar=1.0, in1=dh,
                        op0=mybir.AluOpType.mult, op1=mybir.AluOpType.mult,
                        accum_out=acc[:, col:col + 1])

            nc.sync.dma_start(out=out[:, :NCOL], in_=acc)

    nc.compile()
    return nc


def _get_bass():
    if "nc" not in _CACHE:
        _CACHE["nc"] = _build_bass()
    return _CACHE["nc"]


def _build_tables(vecs, node_ids, elem_ids):
    """Compact (idx, fp8-pair) scatter tables.

    Row mapping sigma: e = 256*kt2 + 128*i + p (kt = e//128, p = e%128);
    col pair (2n, 2n+1) packs into int16 slot n = h*512 + loc
    (little-endian: low byte = col 2n).
    """
    f8 = ml_dtypes.float8_e4m3
    buckets = {}
    for k in range(E2):
        e_row = int(elem_ids[k])
        n = int(node_ids[k])
        key = (e_row, n)
        v = buckets.get(key)
        if v is None:
            buckets[key] = [float(vecs[k, 0]), float(vecs[k, 1])]
        else:
            v[0] += float(vecs[k, 0])
            v[1] += float(vecs[k, 1])
    sidx = np.full((128, KT2, 2, 2, NIDX), -1, dtype=np.int16)
    sval = np.zeros((128, KT2, 2, 2, NIDX), dtype=np.int16)
    fill = np.zeros((128, KT2, 2, 2), dtype=np.int32)
    for (e_row, n), (v0, v1) in buckets.items():
        kt, p = divmod(e_row, 128)
        kt2, i = divmod(kt, 2)
        h, loc = divmod(n, 512)
        j = fill[p, kt2, i, h]
        assert j < NIDX, f"bucket overflow at {(p, kt2, i, h)}"
        fill[p, kt2, i, h] = j + 1
        lo = np.float32(v0).astype(f8).view(np.uint8)
        hi = np.float32(v1).astype(f8).view(np.uint8)
        sidx[p, kt2, i, h, j] = loc
        sval[p, kt2, i, h, j] = np.int16(
            np.uint16(int(lo) | (int(hi) << 8)).view(np.int16))
    return sidx, sval


def _prep_in_maps(EA, e, q, r, vecs, node_ids, elem_ids):
    EA = np.ascontiguousarray(np.asarray(EA, dtype=np.float32))
    e = np.ascontiguousarray(np.asarray(e, dtype=np.float32))
    q = np.ascontiguousarray(np.asarray(q, dtype=np.float32)).reshape(B, N2)
    r = np.ascontiguousarray(np.asarray(r, dtype=np.float32)).reshape(B, N2)
    vecs = np.asarray(vecs, dtype=np.float32)
    sidx, sval = _build_tables(vecs, np.asarray(node_ids), np.asarray(elem_ids))
    ident = np.eye(128, dtype=np.float32).astype(ml_dtypes.bfloat16)
    tabs = np.concatenate([
        sidx.reshape(128, 256),
        sval.reshape(128, 256),
        ident.view(np.int16),
    ], axis=1).astype(np.int16)

    in_maps = []
    for c in range(NCORES):
        sl = slice(c * SHARD, (c + 1) * SHARD)
        in_maps.append({
            "EA": EA[sl], "e": e[sl], "q": q[sl], "r": r[sl],
            "tabs": tabs,
        })
    return in_maps


def _reduce_outs(results):
    total = 0.0
    for c in range(NCORES):
        total += results[c]["out"][:, :NT * NBLK + 1].astype(np.float64).sum()
    return np.array(total / (B * NN * 2), dtype=np.float32)


def kernel_run(EA, e, q, r, vecs, node_ids, elem_ids, trace=False):
    from concourse.bass_utils import run_bass_kernel_spmd

    nc = _get_bass()
    in_maps = _prep_in_maps(EA, e, q, r, vecs, node_ids, elem_ids)
    res = run_bass_kernel_spmd(nc, in_maps, core_ids=list(range(NCORES)),
                               trace=trace)
    return _reduce_outs(res.results), res


def kernel(EA, e, q, r, vecs, node_ids, elem_ids):
    val, _ = kernel_run(EA, e, q, r, vecs, node_ids, elem_ids, trace=False)
    return val
